# revision 35
# baseline (speedup 1.0000x reference)
"""Trainium2 Bass kernel for nn_AttentiveBPNet (grouped attention scoring).

Math (exact algebraic reduction of the reference):
  sk = x @ wk, sv = x @ wv (wk/wv [C,H] folded from W_att/att on host).
  Per group g: score[a,b,t,h] = lrelu(sk[ik(g,a,t),h] + sv[iv(g,b,t),h]),
  mean over t, softmax over b (M=2 -> sigmoid of difference, computed as
  tanh to stay in one ACT table set).

Distribution / algorithm (8 cores, data-parallel over G; no collectives):
  - Each core owns 1024 groups = 16 lanes x 64 groups. The host pre-gathers
    x rows into slot order (host-side indexing prep, like the baseline's
    dedup tables), so the device never gathers: TensorE's matmul columns
    ARE the slots, fed as fp8e4 (host-simulated end-to-end rel err 4.6e-3
    vs 2e-2 tolerance).
  - Column layout: 128 contract rows = 4 sub-slots x 32 channels; two
    accumulating matmuls (channel halves) per 512-col band chunk. Each
    32-partition band (p = 32*B + 8*s + h) receives its own groups directly
    from the matmul (explicit tile_position; out base must be 32-aligned).
  - Per chunk: ACT copies psV -> SBUF bf16 and applies the leaky relu
    (Prelu with a per-partition alpha AP; plain Lrelu's float alpha is
    ignored on HW); DVE forms the (a,b) pair sums via one broadcast
    tensor_tensor (psK PSUM + sbV SBUF) and reduces over t into t2.
    Stages are software-pipelined with full-iteration skews (copy@c,
    TT@c-1, prelu@c-2, red@c-3) so cross-engine semaphore latency never
    paces the pipeline; the last chunk's chain runs entirely on DVE.
  - All input DMA issues go upfront on the compute-free sync queue
    (single queue + large transfers = full 16-SDMA streaming; split
    k/v pieces so K matmuls start on half-arrived chunks).
  - Final: d = t2[b=0]-t2[b=1]; out = 0.5*(1 +- tanh(d*scale)); Copy,
    Prelu and Tanh coexist in one ACT table set -> single table load.
"""

import os

import numpy as np
import ml_dtypes

import concourse.bacc as bacc
import concourse.bass as bass
import concourse.tile as tile
from concourse import mybir, bass_utils

NCORES = 8
N, C, H, M, S, G = 200000, 64, 8, 2, 16, 8192
SLOPE = 0.2
GPC = G // NCORES            # 1024 groups per core
NLANE = 16                   # 4 bands x 4 sub-slots
GPL = GPC // NLANE           # 64 groups per lane
NCH = 8                      # chunks (8 groups per lane each)
GPCH = GPL // NCH            # 16
COLS_B = GPCH * M * S        # 512 cols per band per chunk
COLS = 4 * COLS_B            # cols per chunk (4 bands)
CPM = GPCH * M * M           # t2 cols per chunk

F32 = mybir.dt.float32
BF16 = mybir.dt.bfloat16

USE_FP8 = bool(int(os.environ.get("KERNEL_FP8", "1")))
DT = mybir.dt.float8e4 if USE_FP8 else BF16
NPDT = ml_dtypes.float8_e4m3 if USE_FP8 else ml_dtypes.bfloat16
WSCALE = 32.0 if USE_FP8 else 1.0
TANH_SCALE = 1.0 / (2.0 * S * WSCALE)   # sigmoid(x) = (1+tanh(x/2))/2

_cache: dict = {}


def _build_nc():
    nc = bacc.Bacc(trn_type="TRN2", num_devices=NCORES)
    xin = nc.declare_dram_parameter("xin", [NCH, 128, 4 * COLS], DT,
                                    isOutput=False)
    wts = nc.declare_dram_parameter("wts", [128, 128], DT, isOutput=False)
    yout = nc.declare_dram_parameter("yout", [128, GPL * M * M], F32,
                                     isOutput=True)

    with tile.TileContext(nc) as tc:
        with (
            tc.tile_pool(name="const", bufs=1) as cpool,
            tc.tile_pool(name="xin", bufs=1) as xpool,
            tc.tile_pool(name="psumk", bufs=4, space="PSUM") as ppoolk,
            tc.tile_pool(name="psumv", bufs=3, space="PSUM") as ppoolv,
            tc.tile_pool(name="sb", bufs=3) as spool,
            tc.tile_pool(name="z", bufs=4) as zpool,
            tc.tile_pool(name="acc", bufs=1) as apool,
            tc.tile_pool(name="warm", bufs=1, space="PSUM") as wpool,
        ):
            w_sb = cpool.tile([128, 128], DT)
            alpha_sb = cpool.tile([128, 1], F32)
            nc.vector.memset(alpha_sb[:, :], SLOPE)
            # t2 accumulator: col = (cc*GPCH + jj)*4 + a*2 + b
            t2 = apool.tile([128, GPL * M * M], F32, tag="t2")

            # Distinct per-chunk input tiles, one merged k+v transfer per
            # chunk, all issued upfront on the compute-free sync queue.
            # One queue with large transfers streams all 16 SDMA engines
            # at full rate; two queues contend and run slower.
            CW = 2 * COLS
            x_ts = []
            for cc in range(NCH):
                x_t = xpool.tile([128, 2 * CW], DT, tag=f"x{cc}")
                # k-piece and v-piece separately: K matmuls start as soon
                # as the k half lands
                nc.sync.dma_start(x_t[:, 0:CW], xin[cc, :, 0:CW])
                nc.sync.dma_start(x_t[:, CW : 2 * CW], xin[cc, :, CW : 2 * CW])
                x_ts.append(x_t)
                if cc == 0:
                    # weights are tiny; keep them off the stream head
                    nc.sync.dma_start(w_sb[:, :], wts[:, :])

            # TensorE warm-up: the HAM clock gate keeps the PE at 1.2GHz
            # until ~3.4us of sustained activity. Burn dummy matmuls during
            # the DMA head so real matmuls run at 2.4GHz from chunk 0.
            dummy = cpool.tile([128, 512], BF16, tag="warm")
            nc.gpsimd.memset(dummy[:, :], 0.0)
            psW = wpool.tile([128, 512], F32, tag="psw")
            for _ in range(5):
                nc.tensor.matmul(
                    psW[:, :], lhsT=dummy[:, 0:128], rhs=dummy[:, :],
                    start=True, stop=True,
                )

            # Software-pipelined chunk loop. Cross-engine semaphore
            # latency (~0.3us) makes same-iteration dependency chains
            # pace the whole pipeline, so skew every stage a full
            # iteration apart: copy@c, TT@c-1, prelu@c-2, red@c-3.
            psKs, sbVs, zs, lrs = {}, {}, {}, {}
            for cc in range(NCH + 3):
                if cc < NCH:
                    psK = ppoolk.tile([128, COLS_B], F32, tag="psK")
                    psV = ppoolv.tile([128, COLS_B], F32, tag="psV")
                    psKs[cc] = psK
                    for i, (xt, ps) in enumerate(
                        [(x_ts[cc], psK), (x_ts[cc], psV)]
                    ):
                        for half, st in ((0, True), (1, False)):
                            for B in range(4):
                                lo = CW * i + COLS * half + COLS_B * B
                                nc.tensor.matmul(
                                    ps[32 * B : 32 * B + 32, :],
                                    lhsT=w_sb[
                                        :,
                                        64 * i + 32 * half :
                                        64 * i + 32 * half + 32,
                                    ],
                                    rhs=xt[:, lo : lo + COLS_B],
                                    start=st,
                                    stop=not st,
                                    tile_position=(0, 32 * B),
                                )
                    sbV = spool.tile([128, COLS_B], BF16, tag="sbV")
                    if cc == NCH - 1:
                        nc.vector.tensor_copy(sbV[:, :], psV[:, :])
                    else:
                        nc.scalar.activation(
                            out=sbV[:, :], in_=psV[:, :],
                            func=mybir.ActivationFunctionType.Copy,
                            scale=1.0,
                        )
                    sbVs[cc] = sbV

                if 1 <= cc <= NCH:
                    c1 = cc - 1
                    psK, sbV = psKs.pop(c1), sbVs.pop(c1)
                    kv = psK[:, :].rearrange(
                        "p (j a o t) -> p j a o t", j=GPCH, a=M, o=1
                    )
                    vv = sbV[:, :].rearrange(
                        "p (j o b t) -> p j o b t", j=GPCH, o=1, b=M
                    )
                    kb, vb = bass.broadcast_tensor_aps(kv, vv)
                    z = zpool.tile([128, GPCH * M * M * S], BF16, tag="z")
                    zv = z[:, :].rearrange(
                        "p (j a b t) -> p j a b t", j=GPCH, a=M, b=M
                    )
                    nc.vector.tensor_tensor(
                        out=zv, in0=kb, in1=vb, op=mybir.AluOpType.add,
                    )
                    zs[c1] = z

                if 2 <= cc <= NCH + 1:
                    c2 = cc - 2
                    z = zs.pop(c2)
                    # exact leaky relu: Prelu with per-partition alpha AP
                    # (plain Lrelu's float alpha is ignored on HW -> relu)
                    lr = zpool.tile([128, GPCH * M * M * S], BF16, tag="lr")
                    if c2 == NCH - 1:
                        # exact lrelu on DVE: max(0.2*z, z)
                        nc.vector.scalar_tensor_tensor(
                            out=lr[:, :], in0=z[:, :], scalar=SLOPE,
                            in1=z[:, :], op0=mybir.AluOpType.mult,
                            op1=mybir.AluOpType.max,
                        )
                    else:
                        nc.scalar.activation(
                            out=lr[:, :], in_=z[:, :],
                            func=mybir.ActivationFunctionType.Prelu,
                            scale=1.0, alpha=alpha_sb[:, :],
                        )
                    lrs[c2] = lr

                if cc >= 3:
                    c2 = cc - 3
                    lr = lrs.pop(c2)
                    lrr = lr[:, :].rearrange(
                        "p (q t) -> p q t", q=GPCH * M * M, t=S
                    )
                    nc.vector.tensor_reduce(
                        out=t2[:, CPM * c2 : CPM * (c2 + 1)], in_=lrr,
                        axis=mybir.AxisListType.X, op=mybir.AluOpType.add,
                    )

            t2v = t2[:, :].rearrange("p (ja b) -> p ja b", b=M)
            d = apool.tile([128, GPL * M], F32, tag="d")
            th = apool.tile([128, GPL * M], F32, tag="th")
            out_t = apool.tile([128, GPL * M * M], F32, tag="out")
            ov = out_t[:, :].rearrange("p (ja b) -> p ja b", b=M)
            JA = GPL * M
            for jlo, jhi in [(0, (NCH - 1) * CPM // 2), ((NCH - 1) * CPM // 2, GPL * M * M)]:
                dlo, dhi = jlo // 2, jhi // 2
                nc.vector.tensor_tensor(
                    out=d[:, dlo:dhi], in0=t2v[:, dlo:dhi, 0],
                    in1=t2v[:, dlo:dhi, 1], op=mybir.AluOpType.subtract,
                )
                nc.scalar.activation(
                    out=th[:, dlo:dhi], in_=d[:, dlo:dhi],
                    func=mybir.ActivationFunctionType.Tanh,
                    scale=TANH_SCALE,
                )
                nc.vector.tensor_scalar(
                    out=ov[:, dlo:dhi, 0], in0=th[:, dlo:dhi],
                    scalar1=0.5, scalar2=0.5,
                    op0=mybir.AluOpType.mult, op1=mybir.AluOpType.add,
                )
                nc.vector.tensor_scalar(
                    out=ov[:, dlo:dhi, 1], in0=th[:, dlo:dhi],
                    scalar1=-0.5, scalar2=0.5,
                    op0=mybir.AluOpType.mult, op1=mybir.AluOpType.add,
                )
                nc.sync.dma_start(yout[:, jlo:jhi], out_t[:, jlo:jhi])
    nc.finalize()
    return nc


def _fold_w2(W_att, att):
    Wr = W_att.reshape(C, H, C)
    wk = np.einsum("dhc,hc->dh", Wr, att[:, :C])
    wv = np.einsum("dhc,hc->dh", Wr, att[:, C:])
    return wk.astype(np.float32), wv.astype(np.float32)


def prepare_inputs(x, node_idxes, W_att, att):
    x = np.asarray(x, dtype=np.float32)
    W_att = np.asarray(W_att, dtype=np.float32)
    att = np.asarray(att, dtype=np.float32)
    ni = np.asarray(node_idxes)

    wk, wv = _fold_w2(W_att, att)
    wkq = (wk * WSCALE).astype(NPDT)
    wvq = (wv * WSCALE).astype(NPDT)
    wts = np.zeros((128, 128), dtype=NPDT)
    for s in range(4):
        r = slice(32 * s, 32 * s + 32)
        q = slice(8 * s, 8 * s + 8)
        wts[r, 0:32][:, q] = wkq[0:32]
        wts[r, 32:64][:, q] = wkq[32:64]
        wts[r, 64:96][:, q] = wvq[0:32]
        wts[r, 96:128][:, q] = wvq[32:64]

    xT = np.ascontiguousarray(x.T).astype(NPDT)  # [C, N]

    idx_k = ni[:, :, 1, :]  # [G, M, S] key list (pair index a)
    idx_v = ni[:, :, 0, :]  # [G, M, S] value list (pair index b)

    def build(idx):
        # [G,M,S] -> [core, B, s, cc, jj, a, t] -> gather -> merged buffer
        # [core, cc, 128, 2*COLS] with channel halves side by side
        I = idx.reshape(NCORES, 4, 4, NCH, GPCH, M, S)
        I = I.transpose(0, 3, 2, 1, 4, 5, 6)  # [c, cc, s, B, jj, a, t]
        XG = xT[:, I]  # [C, c, cc, s, B, jj, a, t]
        XG = XG.transpose(1, 2, 3, 0, 4, 5, 6, 7)  # [c, cc, s, C, B,jj,a,t]
        b0 = XG[:, :, :, 0:32].reshape(NCORES, NCH, 128, COLS)
        b1 = XG[:, :, :, 32:64].reshape(NCORES, NCH, 128, COLS)
        return np.ascontiguousarray(
            np.concatenate([b0, b1], axis=3)
        )

    k = build(idx_k)
    v = build(idx_v)
    xin = np.ascontiguousarray(np.concatenate([k, v], axis=3))
    in_maps = []
    for c in range(NCORES):
        in_maps.append({"xin": xin[c], "wts": wts})
    return in_maps


def kernel(x, edge_index, node_idxes, W_att, att, **_unused):
    in_maps = prepare_inputs(x, node_idxes, W_att, att)
    if "nc" not in _cache:
        _cache["nc"] = _build_nc()
    nc = _cache["nc"]

    trace = bool(int(os.environ.get("KERNEL_TRACE", "0")))
    res = bass_utils.run_bass_kernel_spmd(
        nc, in_maps, core_ids=list(range(NCORES)), trace=trace
    )
    _cache["last_result"] = res
    out = np.empty((G, M, M, H), dtype=np.float32)
    for c in range(NCORES):
        y = res.results[c]["yout"]  # [128, GPL*M*M]
        y = y.reshape(4, 4, H, GPL, M, M)     # [B, s, h, j, a, b]
        y = y.transpose(0, 1, 3, 4, 5, 2)     # [B, s, j, a, b, h]
        out[c * GPC : (c + 1) * GPC] = y.reshape(GPC, M, M, H)
    return out
